# revision 22
# baseline (speedup 1.0000x reference)
"""NeuralAdditiveModel TRN2 kernel (v7 — per-feature pipelined + PE warmup).

out[b] = sum_f ( relu(relu(x[b,f]*W1[f,:]+b1[f,:]) @ W2[f] + b2[f]) @ W3[f] + b3[f] ) + bias

Sharding: data-parallel over batch, 8 cores x 1024 rows. No collectives.

Per-core dataflow (matmul operands bf16, PSUM fp32):
  warmup: ~14 back-to-back dummy matmuls during the input-DMA window so the
          PE HAM clock gate reaches 2.4 GHz before the real stream begins.
  z1[f] = K=1 matmul w1row.T @ xrow at row position 32*(f%4)
          -> per-feature psum [128, 512], 5-deep pool (4-way row-tiled packing)
  h1[f] = relu(z1 + b1col) drain (bias folded into drain), ACT/DVE alternating
  z2 pair p=(2p,2p+1): 2 K=128 M=64 matmuls col-tiled at (0,0)/(0,64)
  h2 = relu(z2 + b2col) drain
  out += w3pair.T @ h2: M=1 K=128 col-tiled over 4 positions; L3s are issued
  one 4-pair block late so they pack back-to-back on the PE.
"""

import sys
from contextlib import ExitStack

import numpy as np

sys.path.insert(0, "/opt/trn_rl_repo")

import concourse.bass as bass  # noqa: E402
import concourse.tile as tile  # noqa: E402
from concourse import bacc, mybir  # noqa: E402
from concourse.bass_utils import run_bass_kernel_spmd  # noqa: E402

B, F, S, H1 = 8192, 128, 128, 64
NCORES = 8
BLOC = B // NCORES  # 1024 rows per core
BT = 512            # batch chunk (PSUM bank width in fp32)
NBT = BLOC // BT    # 2
NPAIR = F // 2      # 64 feature pairs
F32 = mybir.dt.float32
BF16 = mybir.dt.bfloat16

L3_POS = 4
N_WARMUP = 14

_CACHE = {}


def _build(l3_pos=L3_POS, n_warmup=N_WARMUP):
    nc = bacc.Bacc(
        "TRN2",
        target_bir_lowering=False,
        debug=False,
        enable_asserts=False,
        num_devices=NCORES,
    )

    xTg_d = nc.dram_tensor("xTg", [4, 32, BLOC], BF16, kind="ExternalInput").ap()
    w1q_d = nc.dram_tensor("w1q", [4, 32 * S], BF16, kind="ExternalInput").ap()
    b1t_d = nc.dram_tensor("b1t", [S, F], F32, kind="ExternalInput").ap()
    w2t_d = nc.dram_tensor("w2t", [S, F * H1], BF16, kind="ExternalInput").ap()
    b2p_d = nc.dram_tensor("b2p", [2 * H1, NPAIR], F32, kind="ExternalInput").ap()
    w3p_d = nc.dram_tensor("w3p", [2 * H1, NPAIR], BF16, kind="ExternalInput").ap()
    nrow_out = 32 * (l3_pos - 1) + 1
    out_d = nc.dram_tensor("out", [NBT, nrow_out, BT], F32, kind="ExternalOutput").ap()

    Relu = mybir.ActivationFunctionType.Relu
    Copy = mybir.ActivationFunctionType.Copy
    ADD = mybir.AluOpType.add
    MAX = mybir.AluOpType.max

    with tile.TileContext(nc) as tc, ExitStack() as ctx:
        singles = ctx.enter_context(tc.tile_pool(name="singles", bufs=1))
        xpool = ctx.enter_context(tc.tile_pool(name="xpool", bufs=2))
        h1p = ctx.enter_context(tc.tile_pool(name="h1p", bufs=14))
        h2p = ctx.enter_context(tc.tile_pool(name="h2p", bufs=14))
        srp = ctx.enter_context(tc.tile_pool(name="srp", bufs=2))
        ps1 = ctx.enter_context(tc.tile_pool(name="ps1", bufs=5, space="PSUM"))
        ps2 = ctx.enter_context(tc.tile_pool(name="ps2", bufs=2, space="PSUM"))
        pso = ctx.enter_context(tc.tile_pool(name="pso", bufs=1, space="PSUM"))

        w1sb = singles.tile([128, 32 * S], BF16)
        b1t = singles.tile([S, F], F32)
        w2sb = singles.tile([S, F * H1], BF16)
        b2p = singles.tile([2 * H1, NPAIR], F32)
        w3p = singles.tile([2 * H1, NPAIR], BF16)

        # ---- PE warmup: dense dummy matmuls while input DMAs are in flight.
        # They write a scratch psum tile sharing the "pout" slot (the first
        # real L3 accumulation starts with start=True, wiping it).
        if n_warmup:
            dumw = singles.tile([128, BT], BF16)
            nc.gpsimd.memset(dumw, 0.25)
            scr = pso.tile([128, BT], F32, tag="pout")
            for _ in range(n_warmup):
                nc.tensor.matmul(
                    out=scr,
                    lhsT=dumw[:, 0:128],
                    rhs=dumw,
                    start=True,
                    stop=True,
                )

        # DMA order: chunk-0 x rows interleaved with w1 rows first (the first
        # L1s need them), then biases/weights, then chunk-1 x rows.
        xaugs = []
        for _ in range(NBT):
            xa = xpool.tile([128, 32 * BT], BF16, tag="xaug")
            xaugs.append(xa)

        def x_dma(bt, i):
            nc.sync.dma_start(
                out=xaugs[bt][32 * i : 32 * i + 1, :].rearrange(
                    "p (g j) -> p g j", g=32
                ),
                in_=xTg_d[i : i + 1, :, bt * BT : (bt + 1) * BT],
            )

        for i in range(4):
            x_dma(0, i)
            nc.sync.dma_start(
                out=w1sb[32 * i : 32 * i + 1, :], in_=w1q_d[i : i + 1, :]
            )
        nc.sync.dma_start(out=b1t, in_=b1t_d)
        nc.sync.dma_start(out=b2p, in_=b2p_d)
        nc.sync.dma_start(out=w3p, in_=w3p_d)
        NW2 = 4
        wslice = F * H1 // NW2
        for j in range(NW2):
            nc.sync.dma_start(
                out=w2sb[:, j * wslice : (j + 1) * wslice],
                in_=w2t_d[:, j * wslice : (j + 1) * wslice],
            )
        for i in range(4):
            x_dma(1, i)

        for bt in range(NBT):
            xaug = xaugs[bt]
            pout = pso.tile([128, BT], F32, tag="pout")

            def emit_l1(f):
                zf = ps1.tile([128, BT], F32, tag="z1")
                r, sl = f % 4, f // 4
                nc.tensor.matmul(
                    out=zf,
                    lhsT=w1sb[32 * r : 32 * r + 1, sl * S : (sl + 1) * S],
                    rhs=xaug[32 * r : 32 * r + 1, sl * BT : (sl + 1) * BT],
                    start=True,
                    stop=True,
                    tile_position=(32 * r, 0),
                )
                return zf

            def emit_h1(f, zf, eng):
                h1 = h1p.tile([128, BT], BF16, tag="h1")
                if eng == "act":
                    nc.scalar.activation(h1, zf, Relu, bias=b1t[:, f : f + 1])
                else:
                    nc.vector.tensor_scalar(h1, zf, b1t[:, f : f + 1], 0.0, ADD, MAX)
                return h1

            def emit_l2(p, h1a, h1b):
                z2 = ps2.tile([128, BT], F32, tag="z2")
                for k, h1 in ((0, h1a), (1, h1b)):
                    f = 2 * p + k
                    nc.tensor.matmul(
                        out=z2[64 * k : 64 * k + 64, :],
                        lhsT=w2sb[:, f * H1 : (f + 1) * H1],
                        rhs=h1,
                        start=True,
                        stop=True,
                        tile_position=(0, 64 * k),
                    )
                return z2

            def emit_h2(p, z2, eng):
                h2 = h2p.tile([128, BT], BF16, tag="h2")
                if eng == "dve":
                    nc.vector.tensor_scalar(h2, z2, b2p[:, p : p + 1], 0.0, ADD, MAX)
                else:
                    nc.scalar.activation(h2, z2, Relu, bias=b2p[:, p : p + 1])
                return h2

            def emit_l3(p, h2):
                pos = 32 * (p % l3_pos)
                nc.tensor.matmul(
                    out=pout[pos : pos + 1, :],
                    lhsT=w3p[:, p : p + 1],
                    rhs=h2,
                    start=(p < l3_pos),
                    stop=(p >= NPAIR - l3_pos),
                    skip_group_check=True,
                    tile_position=(0, pos),
                )

            # Software-pipelined blocks of 4 pairs (8 features, two 4-feature
            # L1 waves for row-tiled packing). Block b's waves interleave
            # with block b-1's L2/h2 work (2 pairs flushed after each wave)
            # and block b-2's L3 quad, so the PE always has ready work while
            # this wave's h1 drains free the z1 slots for the next wave.
            h1s = {}
            l2q = []  # pairs whose h1 tiles are drained and await L2
            l3q = []  # (pair, h2) awaiting L3
            for b in range(NPAIR // 4):
                if bt == 0 and b < 3 and n_warmup:
                    # ramp filler: dependency-free dummies the scheduler can
                    # slot into the early DMA-wait gaps (scr's psum slot is
                    # not touched by real L3s until block 2)
                    for _ in range(2):
                        nc.tensor.matmul(
                            out=scr,
                            lhsT=dumw[:, 0:128],
                            rhs=dumw,
                            start=True,
                            stop=True,
                        )
                for wave in range(2):
                    feats = [8 * b + 4 * wave + i for i in range(4)]
                    zfs = [emit_l1(f) for f in feats]
                    for i, f in enumerate(feats):
                        eng = ("act", "dve")[(f + wave) % 2]
                        h1s[f] = emit_h1(f, zfs[i], eng)
                    for _ in range(2):
                        if l2q:
                            p = l2q.pop(0)
                            z2 = emit_l2(p, h1s.pop(2 * p), h1s.pop(2 * p + 1))
                            l3q.append((p, emit_h2(p, z2, "act" if (p % 2 == 1 or p % 16 == 0) else "dve")))
                if len(l3q) >= 8:
                    for prev_p, prev_h2 in l3q[:4]:
                        emit_l3(prev_p, prev_h2)
                    l3q = l3q[4:]
                l2q += [4 * b + q for q in range(4)]
            for p in l2q:
                z2 = emit_l2(p, h1s.pop(2 * p), h1s.pop(2 * p + 1))
                l3q.append((p, emit_h2(p, z2, "act" if (p % 2 == 1 or p % 16 == 0) else "dve")))
            for prev_p, prev_h2 in l3q:
                emit_l3(prev_p, prev_h2)

            # ---- drain partial rows to SBUF, then DRAM in one block DMA;
            # the host sums rows 0,32,64,96 of each chunk block
            srow = srp.tile([nrow_out, BT], F32, tag="srow")
            nc.scalar.activation(srow, pout[0:nrow_out, :], Copy)
            nc.sync.dma_start(out=out_d[bt : bt + 1, :, :], in_=srow)

    nc.compile()
    return nc


def _prep_shared(W1, b1, W2, b2, W3):
    import ml_dtypes

    bf = ml_dtypes.bfloat16
    w1q = W1.reshape(32, 4, S).transpose(1, 0, 2).reshape(4, 32 * S)
    b1t = np.ascontiguousarray(b1.T)  # [S, F]
    w2t = W2.transpose(1, 0, 2).reshape(S, F * H1)
    b2pm = np.empty((2 * H1, NPAIR), np.float32)
    w3pm = np.empty((2 * H1, NPAIR), np.float32)
    W3f = W3.reshape(F, H1)
    for p in range(NPAIR):
        b2pm[:H1, p] = b2[2 * p]
        b2pm[H1:, p] = b2[2 * p + 1]
        w3pm[:H1, p] = W3f[2 * p]
        w3pm[H1:, p] = W3f[2 * p + 1]
    return {
        "w1q": np.ascontiguousarray(w1q).astype(bf),
        "b1t": b1t.astype(np.float32),
        "w2t": np.ascontiguousarray(w2t).astype(bf),
        "b2p": b2pm,
        "w3p": w3pm.astype(bf),
    }


def _prep_core_inputs(xc, shared):
    import ml_dtypes

    m = dict(shared)
    xT = np.ascontiguousarray(xc.T)  # [F, BLOC]
    m["xTg"] = np.ascontiguousarray(
        xT.reshape(32, 4, BLOC).transpose(1, 0, 2)
    ).astype(ml_dtypes.bfloat16)
    return m


def kernel(x, W1, b1, W2, b2, W3, b3, bias, _trace=False):
    x = np.asarray(x, np.float32)
    W1 = np.asarray(W1, np.float32)
    b1 = np.asarray(b1, np.float32)
    W2 = np.asarray(W2, np.float32)
    b2 = np.asarray(b2, np.float32)
    W3 = np.asarray(W3, np.float32)
    b3 = np.asarray(b3, np.float32)
    bias = np.asarray(bias, np.float32)

    if "nc" not in _CACHE:
        _CACHE["nc"] = _build()
    nc = _CACHE["nc"]

    shared = _prep_shared(W1, b1, W2, b2, W3)
    in_maps = [
        _prep_core_inputs(x[c * BLOC : (c + 1) * BLOC], shared)
        for c in range(NCORES)
    ]

    res = run_bass_kernel_spmd(
        nc, in_maps, core_ids=list(range(NCORES)), trace=_trace
    )
    _CACHE["last_result"] = res

    const = float(b3.sum()) + float(bias.reshape(-1)[0])
    parts = []
    for c in range(NCORES):
        o = res.results[c]["out"]  # [NBT, 97, BT]
        parts.append(o[:, ::32, :].sum(axis=1).reshape(BLOC))
    out = np.concatenate(parts) + const
    return out.reshape(B, 1).astype(np.float32)


# revision 24
# speedup vs baseline: 1.0110x; 1.0110x over previous
"""NeuralAdditiveModel TRN2 kernel (v7 — per-feature pipelined + PE warmup).

out[b] = sum_f ( relu(relu(x[b,f]*W1[f,:]+b1[f,:]) @ W2[f] + b2[f]) @ W3[f] + b3[f] ) + bias

Sharding: data-parallel over batch, 8 cores x 1024 rows. No collectives.

Per-core dataflow (matmul operands bf16, PSUM fp32):
  warmup: ~14 back-to-back dummy matmuls during the input-DMA window so the
          PE HAM clock gate reaches 2.4 GHz before the real stream begins.
  z1[f] = K=1 matmul w1row.T @ xrow at row position 32*(f%4)
          -> per-feature psum [128, 512], 5-deep pool (4-way row-tiled packing)
  h1[f] = relu(z1 + b1col) drain (bias folded into drain), ACT/DVE alternating
  z2 pair p=(2p,2p+1): 2 K=128 M=64 matmuls col-tiled at (0,0)/(0,64)
  h2 = relu(z2 + b2col) drain
  out += w3pair.T @ h2: M=1 K=128 col-tiled over 4 positions; L3s are issued
  one 4-pair block late so they pack back-to-back on the PE.
"""

import sys
from contextlib import ExitStack

import numpy as np

sys.path.insert(0, "/opt/trn_rl_repo")

import concourse.bass as bass  # noqa: E402
import concourse.tile as tile  # noqa: E402
from concourse import bacc, mybir  # noqa: E402
from concourse.bass_utils import run_bass_kernel_spmd  # noqa: E402

B, F, S, H1 = 8192, 128, 128, 64
NCORES = 8
BLOC = B // NCORES  # 1024 rows per core
BT = 512            # batch chunk (PSUM bank width in fp32)
NBT = BLOC // BT    # 2
NPAIR = F // 2      # 64 feature pairs
F32 = mybir.dt.float32
BF16 = mybir.dt.bfloat16

L3_POS = 4
N_WARMUP = 14

_CACHE = {}


def _build(l3_pos=L3_POS, n_warmup=N_WARMUP):
    nc = bacc.Bacc(
        "TRN2",
        target_bir_lowering=False,
        debug=False,
        enable_asserts=False,
        num_devices=NCORES,
    )

    xTg_d = nc.dram_tensor("xTg", [4, 32, BLOC], BF16, kind="ExternalInput").ap()
    w1q_d = nc.dram_tensor("w1q", [4, 32 * S], BF16, kind="ExternalInput").ap()
    b1t_d = nc.dram_tensor("b1t", [S, F], F32, kind="ExternalInput").ap()
    w2t_d = nc.dram_tensor("w2t", [S, F * H1], BF16, kind="ExternalInput").ap()
    b2p_d = nc.dram_tensor("b2p", [2 * H1, NPAIR], F32, kind="ExternalInput").ap()
    w3p_d = nc.dram_tensor("w3p", [2 * H1, NPAIR], BF16, kind="ExternalInput").ap()
    out_d = nc.dram_tensor("out", [NBT * l3_pos, BT], F32, kind="ExternalOutput").ap()

    Relu = mybir.ActivationFunctionType.Relu
    Copy = mybir.ActivationFunctionType.Copy
    ADD = mybir.AluOpType.add
    MAX = mybir.AluOpType.max

    with tile.TileContext(nc) as tc, ExitStack() as ctx:
        singles = ctx.enter_context(tc.tile_pool(name="singles", bufs=1))
        xpool = ctx.enter_context(tc.tile_pool(name="xpool", bufs=2))
        h1p = ctx.enter_context(tc.tile_pool(name="h1p", bufs=14))
        h2p = ctx.enter_context(tc.tile_pool(name="h2p", bufs=14))
        srp = ctx.enter_context(tc.tile_pool(name="srp", bufs=2))
        ps1 = ctx.enter_context(tc.tile_pool(name="ps1", bufs=4, space="PSUM"))
        ps2 = ctx.enter_context(tc.tile_pool(name="ps2", bufs=3, space="PSUM"))
        pso = ctx.enter_context(tc.tile_pool(name="pso", bufs=1, space="PSUM"))

        w1sb = singles.tile([128, 32 * S], BF16)
        b1t = singles.tile([S, F], F32)
        w2sb = singles.tile([S, F * H1], BF16)
        b2p = singles.tile([2 * H1, NPAIR], F32)
        w3p = singles.tile([2 * H1, NPAIR], BF16)

        # ---- PE warmup: dense dummy matmuls while input DMAs are in flight.
        # They write a scratch psum tile sharing the "pout" slot (the first
        # real L3 accumulation starts with start=True, wiping it).
        if n_warmup:
            dumw = singles.tile([128, BT], BF16)
            nc.gpsimd.memset(dumw, 0.25)
            scr = pso.tile([128, BT], F32, tag="pout")
            for _ in range(n_warmup):
                nc.tensor.matmul(
                    out=scr,
                    lhsT=dumw[:, 0:128],
                    rhs=dumw,
                    start=True,
                    stop=True,
                )

        # DMA order: chunk-0 x rows interleaved with w1 rows first (the first
        # L1s need them), then biases/weights, then chunk-1 x rows.
        xaugs = []
        for _ in range(NBT):
            xa = xpool.tile([128, 32 * BT], BF16, tag="xaug")
            xaugs.append(xa)

        def x_dma(bt, i):
            nc.sync.dma_start(
                out=xaugs[bt][32 * i : 32 * i + 1, :].rearrange(
                    "p (g j) -> p g j", g=32
                ),
                in_=xTg_d[i : i + 1, :, bt * BT : (bt + 1) * BT],
            )

        for i in range(4):
            x_dma(0, i)
            nc.sync.dma_start(
                out=w1sb[32 * i : 32 * i + 1, :], in_=w1q_d[i : i + 1, :]
            )
        nc.sync.dma_start(out=b1t, in_=b1t_d)
        nc.sync.dma_start(out=b2p, in_=b2p_d)
        nc.sync.dma_start(out=w3p, in_=w3p_d)
        NW2 = 4
        wslice = F * H1 // NW2
        for j in range(NW2):
            nc.sync.dma_start(
                out=w2sb[:, j * wslice : (j + 1) * wslice],
                in_=w2t_d[:, j * wslice : (j + 1) * wslice],
            )
        for i in range(4):
            x_dma(1, i)

        for bt in range(NBT):
            xaug = xaugs[bt]
            pout = pso.tile([128, BT], F32, tag="pout")

            def emit_l1(f):
                zf = ps1.tile([128, BT], F32, tag="z1")
                r, sl = f % 4, f // 4
                nc.tensor.matmul(
                    out=zf,
                    lhsT=w1sb[32 * r : 32 * r + 1, sl * S : (sl + 1) * S],
                    rhs=xaug[32 * r : 32 * r + 1, sl * BT : (sl + 1) * BT],
                    start=True,
                    stop=True,
                    tile_position=(32 * r, 0),
                )
                return zf

            def emit_h1(f, zf, eng):
                h1 = h1p.tile([128, BT], BF16, tag="h1")
                if eng == "act":
                    nc.scalar.activation(h1, zf, Relu, bias=b1t[:, f : f + 1])
                else:
                    nc.vector.tensor_scalar(h1, zf, b1t[:, f : f + 1], 0.0, ADD, MAX)
                return h1

            def emit_l2(p, h1a, h1b):
                z2 = ps2.tile([128, BT], F32, tag="z2")
                for k, h1 in ((0, h1a), (1, h1b)):
                    f = 2 * p + k
                    nc.tensor.matmul(
                        out=z2[64 * k : 64 * k + 64, :],
                        lhsT=w2sb[:, f * H1 : (f + 1) * H1],
                        rhs=h1,
                        start=True,
                        stop=True,
                        tile_position=(0, 64 * k),
                    )
                return z2

            def emit_h2(p, z2, eng):
                h2 = h2p.tile([128, BT], BF16, tag="h2")
                if eng == "dve":
                    nc.vector.tensor_scalar(h2, z2, b2p[:, p : p + 1], 0.0, ADD, MAX)
                else:
                    nc.scalar.activation(h2, z2, Relu, bias=b2p[:, p : p + 1])
                return h2

            def emit_l3(p, h2):
                pos = 32 * (p % l3_pos)
                nc.tensor.matmul(
                    out=pout[pos : pos + 1, :],
                    lhsT=w3p[:, p : p + 1],
                    rhs=h2,
                    start=(p < l3_pos),
                    stop=(p >= NPAIR - l3_pos),
                    skip_group_check=True,
                    tile_position=(0, pos),
                )

            # Software-pipelined blocks of 4 pairs (8 features, two 4-feature
            # L1 waves for row-tiled packing). Block b's waves interleave
            # with block b-1's L2/h2 work (2 pairs flushed after each wave)
            # and block b-2's L3 quad, so the PE always has ready work while
            # this wave's h1 drains free the z1 slots for the next wave.
            h1s = {}
            l2q = []  # pairs whose h1 tiles are drained and await L2
            l3q = []  # (pair, h2) awaiting L3
            for b in range(NPAIR // 4):
                if bt == 0 and b < 3 and n_warmup:
                    # ramp filler: dependency-free dummies the scheduler can
                    # slot into the early DMA-wait gaps (scr's psum slot is
                    # not touched by real L3s until block 2)
                    for _ in range(2):
                        nc.tensor.matmul(
                            out=scr,
                            lhsT=dumw[:, 0:128],
                            rhs=dumw,
                            start=True,
                            stop=True,
                        )
                for wave in range(2):
                    feats = [8 * b + 4 * wave + i for i in range(4)]
                    zfs = [emit_l1(f) for f in feats]
                    for i, f in enumerate(feats):
                        eng = ("act", "dve")[(f + wave) % 2]
                        h1s[f] = emit_h1(f, zfs[i], eng)
                    for _ in range(2):
                        if l2q:
                            p = l2q.pop(0)
                            z2 = emit_l2(p, h1s.pop(2 * p), h1s.pop(2 * p + 1))
                            l3q.append((p, emit_h2(p, z2, "act" if (p % 2 == 1 or p % 16 == 0) else "dve")))
                if len(l3q) >= 8:
                    for prev_p, prev_h2 in l3q[:4]:
                        emit_l3(prev_p, prev_h2)
                    l3q = l3q[4:]
                l2q += [4 * b + q for q in range(4)]
            for p in l2q:
                z2 = emit_l2(p, h1s.pop(2 * p), h1s.pop(2 * p + 1))
                l3q.append((p, emit_h2(p, z2, "act" if (p % 2 == 1 or p % 16 == 0) else "dve")))
            for prev_p, prev_h2 in l3q:
                emit_l3(prev_p, prev_h2)

            # ---- drain partial rows to SBUF, then DRAM; host sums them
            nrow = 32 * (l3_pos - 1) + 1
            srow = srp.tile([nrow, BT], F32, tag="srow")
            nc.scalar.activation(srow, pout[0:nrow, :], Copy)
            for k in range(l3_pos):
                nc.sync.dma_start(
                    out=out_d[l3_pos * bt + k : l3_pos * bt + k + 1, :],
                    in_=srow[32 * k : 32 * k + 1, :],
                )

    nc.compile()
    return nc


def _prep_shared(W1, b1, W2, b2, W3):
    import ml_dtypes

    bf = ml_dtypes.bfloat16
    w1q = W1.reshape(32, 4, S).transpose(1, 0, 2).reshape(4, 32 * S)
    b1t = np.ascontiguousarray(b1.T)  # [S, F]
    w2t = W2.transpose(1, 0, 2).reshape(S, F * H1)
    b2pm = np.empty((2 * H1, NPAIR), np.float32)
    w3pm = np.empty((2 * H1, NPAIR), np.float32)
    W3f = W3.reshape(F, H1)
    for p in range(NPAIR):
        b2pm[:H1, p] = b2[2 * p]
        b2pm[H1:, p] = b2[2 * p + 1]
        w3pm[:H1, p] = W3f[2 * p]
        w3pm[H1:, p] = W3f[2 * p + 1]
    return {
        "w1q": np.ascontiguousarray(w1q).astype(bf),
        "b1t": b1t.astype(np.float32),
        "w2t": np.ascontiguousarray(w2t).astype(bf),
        "b2p": b2pm,
        "w3p": w3pm.astype(bf),
    }


def _prep_core_inputs(xc, shared):
    import ml_dtypes

    m = dict(shared)
    xT = np.ascontiguousarray(xc.T)  # [F, BLOC]
    m["xTg"] = np.ascontiguousarray(
        xT.reshape(32, 4, BLOC).transpose(1, 0, 2)
    ).astype(ml_dtypes.bfloat16)
    return m


def kernel(x, W1, b1, W2, b2, W3, b3, bias, _trace=False):
    x = np.asarray(x, np.float32)
    W1 = np.asarray(W1, np.float32)
    b1 = np.asarray(b1, np.float32)
    W2 = np.asarray(W2, np.float32)
    b2 = np.asarray(b2, np.float32)
    W3 = np.asarray(W3, np.float32)
    b3 = np.asarray(b3, np.float32)
    bias = np.asarray(bias, np.float32)

    if "nc" not in _CACHE:
        _CACHE["nc"] = _build()
    nc = _CACHE["nc"]

    shared = _prep_shared(W1, b1, W2, b2, W3)
    in_maps = [
        _prep_core_inputs(x[c * BLOC : (c + 1) * BLOC], shared)
        for c in range(NCORES)
    ]

    res = run_bass_kernel_spmd(
        nc, in_maps, core_ids=list(range(NCORES)), trace=_trace
    )
    _CACHE["last_result"] = res

    const = float(b3.sum()) + float(bias.reshape(-1)[0])
    parts = []
    for c in range(NCORES):
        o = res.results[c]["out"]  # [NBT*L3_POS, BT]
        parts.append(o.reshape(NBT, L3_POS, BT).sum(axis=1).reshape(BLOC))
    out = np.concatenate(parts) + const
    return out.reshape(B, 1).astype(np.float32)


# revision 25
# speedup vs baseline: 1.0170x; 1.0059x over previous
"""NeuralAdditiveModel TRN2 kernel (v7 — per-feature pipelined + PE warmup).

out[b] = sum_f ( relu(relu(x[b,f]*W1[f,:]+b1[f,:]) @ W2[f] + b2[f]) @ W3[f] + b3[f] ) + bias

Sharding: data-parallel over batch, 8 cores x 1024 rows. No collectives.

Per-core dataflow (matmul operands bf16, PSUM fp32):
  warmup: ~14 back-to-back dummy matmuls during the input-DMA window so the
          PE HAM clock gate reaches 2.4 GHz before the real stream begins.
  z1[f] = K=1 matmul w1row.T @ xrow at row position 32*(f%4)
          -> per-feature psum [128, 512], 5-deep pool (4-way row-tiled packing)
  h1[f] = relu(z1 + b1col) drain (bias folded into drain), ACT/DVE alternating
  z2 pair p=(2p,2p+1): 2 K=128 M=64 matmuls col-tiled at (0,0)/(0,64)
  h2 = relu(z2 + b2col) drain
  out += w3pair.T @ h2: M=1 K=128 col-tiled over 4 positions; L3s are issued
  one 4-pair block late so they pack back-to-back on the PE.
"""

import sys
from contextlib import ExitStack

import numpy as np

sys.path.insert(0, "/opt/trn_rl_repo")

import concourse.bass as bass  # noqa: E402
import concourse.tile as tile  # noqa: E402
from concourse import bacc, mybir  # noqa: E402
from concourse.bass_utils import run_bass_kernel_spmd  # noqa: E402

B, F, S, H1 = 8192, 128, 128, 64
NCORES = 8
BLOC = B // NCORES  # 1024 rows per core
BT = 512            # batch chunk (PSUM bank width in fp32)
NBT = BLOC // BT    # 2
NPAIR = F // 2      # 64 feature pairs
F32 = mybir.dt.float32
BF16 = mybir.dt.bfloat16

L3_POS = 4
N_WARMUP = 14

_CACHE = {}


def _build(l3_pos=L3_POS, n_warmup=N_WARMUP):
    nc = bacc.Bacc(
        "TRN2",
        target_bir_lowering=False,
        debug=False,
        enable_asserts=False,
        num_devices=NCORES,
    )

    xTg_d = nc.dram_tensor("xTg", [4, 32, BLOC], BF16, kind="ExternalInput").ap()
    w1q_d = nc.dram_tensor("w1q", [4, 32 * S], BF16, kind="ExternalInput").ap()
    b1t_d = nc.dram_tensor("b1t", [S, F], F32, kind="ExternalInput").ap()
    w2t_d = nc.dram_tensor("w2t", [S, F * H1], BF16, kind="ExternalInput").ap()
    b2p_d = nc.dram_tensor("b2p", [2 * H1, NPAIR], F32, kind="ExternalInput").ap()
    w3p_d = nc.dram_tensor("w3p", [2 * H1, NPAIR], BF16, kind="ExternalInput").ap()
    out_d = nc.dram_tensor("out", [NBT * l3_pos, BT], F32, kind="ExternalOutput").ap()

    Relu = mybir.ActivationFunctionType.Relu
    Copy = mybir.ActivationFunctionType.Copy
    ADD = mybir.AluOpType.add
    MAX = mybir.AluOpType.max

    with tile.TileContext(nc) as tc, ExitStack() as ctx:
        singles = ctx.enter_context(tc.tile_pool(name="singles", bufs=1))
        xpool = ctx.enter_context(tc.tile_pool(name="xpool", bufs=2))
        h1p = ctx.enter_context(tc.tile_pool(name="h1p", bufs=14))
        h2p = ctx.enter_context(tc.tile_pool(name="h2p", bufs=14))
        srp = ctx.enter_context(tc.tile_pool(name="srp", bufs=2))
        ps1 = ctx.enter_context(tc.tile_pool(name="ps1", bufs=5, space="PSUM"))
        ps2 = ctx.enter_context(tc.tile_pool(name="ps2", bufs=2, space="PSUM"))
        pso = ctx.enter_context(tc.tile_pool(name="pso", bufs=1, space="PSUM"))

        w1sb = singles.tile([128, 32 * S], BF16)
        b1t = singles.tile([S, F], F32)
        w2sb = singles.tile([S, F * H1], BF16)
        b2p = singles.tile([2 * H1, NPAIR], F32)
        w3p = singles.tile([2 * H1, NPAIR], BF16)

        # ---- PE warmup: dense dummy matmuls while input DMAs are in flight.
        # They write a scratch psum tile sharing the "pout" slot (the first
        # real L3 accumulation starts with start=True, wiping it).
        if n_warmup:
            dumw = singles.tile([128, BT], BF16)
            nc.gpsimd.memset(dumw, 0.25)
            scr = pso.tile([128, BT], F32, tag="pout")
            for _ in range(n_warmup):
                nc.tensor.matmul(
                    out=scr,
                    lhsT=dumw[:, 0:128],
                    rhs=dumw,
                    start=True,
                    stop=True,
                )

        # DMA order: chunk-0 x rows interleaved with w1 rows first (the first
        # L1s need them), then biases/weights, then chunk-1 x rows.
        xaugs = []
        for _ in range(NBT):
            xa = xpool.tile([128, 32 * BT], BF16, tag="xaug")
            xaugs.append(xa)

        def x_dma(bt, i):
            nc.sync.dma_start(
                out=xaugs[bt][32 * i : 32 * i + 1, :].rearrange(
                    "p (g j) -> p g j", g=32
                ),
                in_=xTg_d[i : i + 1, :, bt * BT : (bt + 1) * BT],
            )

        for i in range(4):
            x_dma(0, i)
            nc.sync.dma_start(
                out=w1sb[32 * i : 32 * i + 1, :], in_=w1q_d[i : i + 1, :]
            )
        nc.sync.dma_start(out=b1t, in_=b1t_d)
        nc.sync.dma_start(out=b2p, in_=b2p_d)
        nc.sync.dma_start(out=w3p, in_=w3p_d)
        NW2 = 4
        wslice = F * H1 // NW2
        for j in range(NW2):
            nc.sync.dma_start(
                out=w2sb[:, j * wslice : (j + 1) * wslice],
                in_=w2t_d[:, j * wslice : (j + 1) * wslice],
            )
        for i in range(4):
            x_dma(1, i)

        for bt in range(NBT):
            xaug = xaugs[bt]
            pout = pso.tile([128, BT], F32, tag="pout")

            def emit_l1(f):
                zf = ps1.tile([128, BT], F32, tag="z1")
                r, sl = f % 4, f // 4
                nc.tensor.matmul(
                    out=zf,
                    lhsT=w1sb[32 * r : 32 * r + 1, sl * S : (sl + 1) * S],
                    rhs=xaug[32 * r : 32 * r + 1, sl * BT : (sl + 1) * BT],
                    start=True,
                    stop=True,
                    tile_position=(32 * r, 0),
                )
                return zf

            def emit_h1(f, zf, eng):
                h1 = h1p.tile([128, BT], BF16, tag="h1")
                if eng == "act":
                    nc.scalar.activation(h1, zf, Relu, bias=b1t[:, f : f + 1])
                else:
                    nc.vector.tensor_scalar(h1, zf, b1t[:, f : f + 1], 0.0, ADD, MAX)
                return h1

            def emit_l2(p, h1a, h1b):
                z2 = ps2.tile([128, BT], F32, tag="z2")
                for k, h1 in ((0, h1a), (1, h1b)):
                    f = 2 * p + k
                    nc.tensor.matmul(
                        out=z2[64 * k : 64 * k + 64, :],
                        lhsT=w2sb[:, f * H1 : (f + 1) * H1],
                        rhs=h1,
                        start=True,
                        stop=True,
                        tile_position=(0, 64 * k),
                    )
                return z2

            def emit_h2(p, z2, eng):
                h2 = h2p.tile([128, BT], BF16, tag="h2")
                if eng == "dve":
                    nc.vector.tensor_scalar(h2, z2, b2p[:, p : p + 1], 0.0, ADD, MAX)
                else:
                    nc.scalar.activation(h2, z2, Relu, bias=b2p[:, p : p + 1])
                return h2

            def emit_l3(p, h2):
                pos = 32 * (p % l3_pos)
                nc.tensor.matmul(
                    out=pout[pos : pos + 1, :],
                    lhsT=w3p[:, p : p + 1],
                    rhs=h2,
                    start=(p < l3_pos),
                    stop=(p >= NPAIR - l3_pos),
                    skip_group_check=True,
                    tile_position=(0, pos),
                )

            # Software-pipelined blocks of 4 pairs (8 features, two 4-feature
            # L1 waves for row-tiled packing). Block b's waves interleave
            # with block b-1's L2/h2 work (2 pairs flushed after each wave)
            # and block b-2's L3 quad, so the PE always has ready work while
            # this wave's h1 drains free the z1 slots for the next wave.
            h1s = {}
            l2q = []  # pairs whose h1 tiles are drained and await L2
            l3q = []  # (pair, h2) awaiting L3
            for b in range(NPAIR // 4):
                if bt == 0 and b < 3 and n_warmup:
                    # ramp filler: dependency-free dummies the scheduler can
                    # slot into the early DMA-wait gaps (scr's psum slot is
                    # not touched by real L3s until block 2)
                    for _ in range(2):
                        nc.tensor.matmul(
                            out=scr,
                            lhsT=dumw[:, 0:128],
                            rhs=dumw,
                            start=True,
                            stop=True,
                        )
                for wave in range(2):
                    feats = [8 * b + 4 * wave + i for i in range(4)]
                    zfs = [emit_l1(f) for f in feats]
                    for i, f in enumerate(feats):
                        eng = ("act", "dve")[(f + wave) % 2]
                        h1s[f] = emit_h1(f, zfs[i], eng)
                    for _ in range(2):
                        if l2q:
                            p = l2q.pop(0)
                            z2 = emit_l2(p, h1s.pop(2 * p), h1s.pop(2 * p + 1))
                            l3q.append((p, emit_h2(p, z2, "act" if (p % 2 == 1 or p % 16 == 0) else "dve")))
                if len(l3q) >= 8:
                    for prev_p, prev_h2 in l3q[:4]:
                        emit_l3(prev_p, prev_h2)
                    l3q = l3q[4:]
                l2q += [4 * b + q for q in range(4)]
            for p in l2q:
                z2 = emit_l2(p, h1s.pop(2 * p), h1s.pop(2 * p + 1))
                l3q.append((p, emit_h2(p, z2, "act" if (p % 2 == 1 or p % 16 == 0) else "dve")))
            for prev_p, prev_h2 in l3q:
                emit_l3(prev_p, prev_h2)

            # ---- drain partial rows to SBUF, then DRAM; host sums them
            nrow = 32 * (l3_pos - 1) + 1
            srow = srp.tile([nrow, BT], F32, tag="srow")
            nc.scalar.activation(srow, pout[0:nrow, :], Copy)
            for k in range(l3_pos):
                nc.sync.dma_start(
                    out=out_d[l3_pos * bt + k : l3_pos * bt + k + 1, :],
                    in_=srow[32 * k : 32 * k + 1, :],
                )

    nc.compile()
    return nc


def _prep_shared(W1, b1, W2, b2, W3):
    import ml_dtypes

    bf = ml_dtypes.bfloat16
    w1q = W1.reshape(32, 4, S).transpose(1, 0, 2).reshape(4, 32 * S)
    b1t = np.ascontiguousarray(b1.T)  # [S, F]
    w2t = W2.transpose(1, 0, 2).reshape(S, F * H1)
    b2pm = np.empty((2 * H1, NPAIR), np.float32)
    w3pm = np.empty((2 * H1, NPAIR), np.float32)
    W3f = W3.reshape(F, H1)
    for p in range(NPAIR):
        b2pm[:H1, p] = b2[2 * p]
        b2pm[H1:, p] = b2[2 * p + 1]
        w3pm[:H1, p] = W3f[2 * p]
        w3pm[H1:, p] = W3f[2 * p + 1]
    return {
        "w1q": np.ascontiguousarray(w1q).astype(bf),
        "b1t": b1t.astype(np.float32),
        "w2t": np.ascontiguousarray(w2t).astype(bf),
        "b2p": b2pm,
        "w3p": w3pm.astype(bf),
    }


def _prep_core_inputs(xc, shared):
    import ml_dtypes

    m = dict(shared)
    xT = np.ascontiguousarray(xc.T)  # [F, BLOC]
    m["xTg"] = np.ascontiguousarray(
        xT.reshape(32, 4, BLOC).transpose(1, 0, 2)
    ).astype(ml_dtypes.bfloat16)
    return m


def kernel(x, W1, b1, W2, b2, W3, b3, bias, _trace=False):
    x = np.asarray(x, np.float32)
    W1 = np.asarray(W1, np.float32)
    b1 = np.asarray(b1, np.float32)
    W2 = np.asarray(W2, np.float32)
    b2 = np.asarray(b2, np.float32)
    W3 = np.asarray(W3, np.float32)
    b3 = np.asarray(b3, np.float32)
    bias = np.asarray(bias, np.float32)

    if "nc" not in _CACHE:
        _CACHE["nc"] = _build()
    nc = _CACHE["nc"]

    shared = _prep_shared(W1, b1, W2, b2, W3)
    in_maps = [
        _prep_core_inputs(x[c * BLOC : (c + 1) * BLOC], shared)
        for c in range(NCORES)
    ]

    res = run_bass_kernel_spmd(
        nc, in_maps, core_ids=list(range(NCORES)), trace=_trace
    )
    _CACHE["last_result"] = res

    const = float(b3.sum()) + float(bias.reshape(-1)[0])
    parts = []
    for c in range(NCORES):
        o = res.results[c]["out"]  # [NBT*L3_POS, BT]
        parts.append(o.reshape(NBT, L3_POS, BT).sum(axis=1).reshape(BLOC))
    out = np.concatenate(parts) + const
    return out.reshape(B, 1).astype(np.float32)


# revision 26
# speedup vs baseline: 1.0407x; 1.0233x over previous
"""NeuralAdditiveModel TRN2 kernel (v7 — per-feature pipelined + PE warmup).

out[b] = sum_f ( relu(relu(x[b,f]*W1[f,:]+b1[f,:]) @ W2[f] + b2[f]) @ W3[f] + b3[f] ) + bias

Sharding: data-parallel over batch, 8 cores x 1024 rows. No collectives.

Per-core dataflow (matmul operands bf16, PSUM fp32):
  warmup: ~14 back-to-back dummy matmuls during the input-DMA window so the
          PE HAM clock gate reaches 2.4 GHz before the real stream begins.
  z1[f] = K=1 matmul w1row.T @ xrow at row position 32*(f%4)
          -> per-feature psum [128, 512], 5-deep pool (4-way row-tiled packing)
  h1[f] = relu(z1 + b1col) drain (bias folded into drain), ACT/DVE alternating
  z2 pair p=(2p,2p+1): 2 K=128 M=64 matmuls col-tiled at (0,0)/(0,64)
  h2 = relu(z2 + b2col) drain
  out += w3pair.T @ h2: M=1 K=128 col-tiled over 4 positions; L3s are issued
  one 4-pair block late so they pack back-to-back on the PE.
"""

import sys
from contextlib import ExitStack

import numpy as np

sys.path.insert(0, "/opt/trn_rl_repo")

import concourse.bass as bass  # noqa: E402
import concourse.tile as tile  # noqa: E402
from concourse import bacc, mybir  # noqa: E402
from concourse.bass_utils import run_bass_kernel_spmd  # noqa: E402

B, F, S, H1 = 8192, 128, 128, 64
NCORES = 8
BLOC = B // NCORES  # 1024 rows per core
BT = 512            # batch chunk (PSUM bank width in fp32)
NBT = BLOC // BT    # 2
NPAIR = F // 2      # 64 feature pairs
F32 = mybir.dt.float32
BF16 = mybir.dt.bfloat16

L3_POS = 4
N_WARMUP = 14

_CACHE = {}


def _build(l3_pos=L3_POS, n_warmup=N_WARMUP):
    nc = bacc.Bacc(
        "TRN2",
        target_bir_lowering=False,
        debug=False,
        enable_asserts=False,
        num_devices=NCORES,
    )

    xTg_d = nc.dram_tensor("xTg", [4, 32, BLOC], BF16, kind="ExternalInput").ap()
    w1q_d = nc.dram_tensor("w1q", [4, 32 * S], BF16, kind="ExternalInput").ap()
    b1t_d = nc.dram_tensor("b1t", [S, F], F32, kind="ExternalInput").ap()
    w2t_d = nc.dram_tensor("w2t", [S, F * H1], BF16, kind="ExternalInput").ap()
    b2p_d = nc.dram_tensor("b2p", [2 * H1, NPAIR], F32, kind="ExternalInput").ap()
    w3p_d = nc.dram_tensor("w3p", [2 * H1, NPAIR], BF16, kind="ExternalInput").ap()
    out_d = nc.dram_tensor("out", [NBT * l3_pos, BT], F32, kind="ExternalOutput").ap()

    Relu = mybir.ActivationFunctionType.Relu
    Copy = mybir.ActivationFunctionType.Copy
    ADD = mybir.AluOpType.add
    MAX = mybir.AluOpType.max

    with tile.TileContext(nc) as tc, ExitStack() as ctx:
        singles = ctx.enter_context(tc.tile_pool(name="singles", bufs=1))
        xpool = ctx.enter_context(tc.tile_pool(name="xpool", bufs=2))
        h1p = ctx.enter_context(tc.tile_pool(name="h1p", bufs=14))
        h2p = ctx.enter_context(tc.tile_pool(name="h2p", bufs=14))
        srp = ctx.enter_context(tc.tile_pool(name="srp", bufs=2))
        ps1 = ctx.enter_context(tc.tile_pool(name="ps1", bufs=5, space="PSUM"))
        ps2 = ctx.enter_context(tc.tile_pool(name="ps2", bufs=2, space="PSUM"))
        pso = ctx.enter_context(tc.tile_pool(name="pso", bufs=1, space="PSUM"))

        w1sb = singles.tile([128, 32 * S], BF16)
        b1t = singles.tile([S, F], F32)
        w2sb = singles.tile([S, F * H1], BF16)
        b2p = singles.tile([2 * H1, NPAIR], F32)
        w3p = singles.tile([2 * H1, NPAIR], BF16)

        # ---- PE warmup: dense dummy matmuls while input DMAs are in flight.
        # They write a scratch psum tile sharing the "pout" slot (the first
        # real L3 accumulation starts with start=True, wiping it).
        if n_warmup:
            dumw = singles.tile([128, BT], BF16)
            nc.gpsimd.memset(dumw, 0.25)
            scr = pso.tile([128, BT], F32, tag="pout")
            for _ in range(n_warmup):
                nc.tensor.matmul(
                    out=scr,
                    lhsT=dumw[:, 0:128],
                    rhs=dumw,
                    start=True,
                    stop=True,
                )

        # DMA order: chunk-0 x rows interleaved with w1 rows first (the first
        # L1s need them), then biases/weights, then chunk-1 x rows.
        xaugs = []
        for _ in range(NBT):
            xa = xpool.tile([128, 32 * BT], BF16, tag="xaug")
            xaugs.append(xa)

        def x_dma(bt, i):
            nc.sync.dma_start(
                out=xaugs[bt][32 * i : 32 * i + 1, :].rearrange(
                    "p (g j) -> p g j", g=32
                ),
                in_=xTg_d[i : i + 1, :, bt * BT : (bt + 1) * BT],
            )

        for i in range(4):
            x_dma(0, i)
            nc.sync.dma_start(
                out=w1sb[32 * i : 32 * i + 1, :], in_=w1q_d[i : i + 1, :]
            )
        nc.gpsimd.dma_start(out=b1t, in_=b1t_d)
        nc.gpsimd.dma_start(out=b2p, in_=b2p_d)
        nc.gpsimd.dma_start(out=w3p, in_=w3p_d)
        NW2 = 4
        wslice = F * H1 // NW2
        for j in range(NW2):
            nc.gpsimd.dma_start(
                out=w2sb[:, j * wslice : (j + 1) * wslice],
                in_=w2t_d[:, j * wslice : (j + 1) * wslice],
            )
        for i in range(4):
            x_dma(1, i)

        for bt in range(NBT):
            xaug = xaugs[bt]
            pout = pso.tile([128, BT], F32, tag="pout")

            def emit_l1(f):
                zf = ps1.tile([128, BT], F32, tag="z1")
                r, sl = f % 4, f // 4
                nc.tensor.matmul(
                    out=zf,
                    lhsT=w1sb[32 * r : 32 * r + 1, sl * S : (sl + 1) * S],
                    rhs=xaug[32 * r : 32 * r + 1, sl * BT : (sl + 1) * BT],
                    start=True,
                    stop=True,
                    tile_position=(32 * r, 0),
                )
                return zf

            def emit_h1(f, zf, eng):
                h1 = h1p.tile([128, BT], BF16, tag="h1")
                if eng == "act":
                    nc.scalar.activation(h1, zf, Relu, bias=b1t[:, f : f + 1])
                else:
                    nc.vector.tensor_scalar(h1, zf, b1t[:, f : f + 1], 0.0, ADD, MAX)
                return h1

            def emit_l2(p, h1a, h1b):
                z2 = ps2.tile([128, BT], F32, tag="z2")
                for k, h1 in ((0, h1a), (1, h1b)):
                    f = 2 * p + k
                    nc.tensor.matmul(
                        out=z2[64 * k : 64 * k + 64, :],
                        lhsT=w2sb[:, f * H1 : (f + 1) * H1],
                        rhs=h1,
                        start=True,
                        stop=True,
                        tile_position=(0, 64 * k),
                    )
                return z2

            def emit_h2(p, z2, eng):
                h2 = h2p.tile([128, BT], BF16, tag="h2")
                if eng == "dve":
                    nc.vector.tensor_scalar(h2, z2, b2p[:, p : p + 1], 0.0, ADD, MAX)
                else:
                    nc.scalar.activation(h2, z2, Relu, bias=b2p[:, p : p + 1])
                return h2

            def emit_l3(p, h2):
                pos = 32 * (p % l3_pos)
                nc.tensor.matmul(
                    out=pout[pos : pos + 1, :],
                    lhsT=w3p[:, p : p + 1],
                    rhs=h2,
                    start=(p < l3_pos),
                    stop=(p >= NPAIR - l3_pos),
                    skip_group_check=True,
                    tile_position=(0, pos),
                )

            # Software-pipelined blocks of 4 pairs (8 features, two 4-feature
            # L1 waves for row-tiled packing). Block b's waves interleave
            # with block b-1's L2/h2 work (2 pairs flushed after each wave)
            # and block b-2's L3 quad, so the PE always has ready work while
            # this wave's h1 drains free the z1 slots for the next wave.
            h1s = {}
            l2q = []  # pairs whose h1 tiles are drained and await L2
            l3q = []  # (pair, h2) awaiting L3
            for b in range(NPAIR // 4):
                if bt == 0 and b < 3 and n_warmup:
                    # ramp filler: dependency-free dummies the scheduler can
                    # slot into the early DMA-wait gaps (scr's psum slot is
                    # not touched by real L3s until block 2)
                    for _ in range(2):
                        nc.tensor.matmul(
                            out=scr,
                            lhsT=dumw[:, 0:128],
                            rhs=dumw,
                            start=True,
                            stop=True,
                        )
                for wave in range(2):
                    feats = [8 * b + 4 * wave + i for i in range(4)]
                    zfs = [emit_l1(f) for f in feats]
                    for i, f in enumerate(feats):
                        eng = ("act", "dve")[(f + wave) % 2]
                        h1s[f] = emit_h1(f, zfs[i], eng)
                    for _ in range(2):
                        if l2q:
                            p = l2q.pop(0)
                            z2 = emit_l2(p, h1s.pop(2 * p), h1s.pop(2 * p + 1))
                            l3q.append((p, emit_h2(p, z2, "act" if (p % 2 == 1 or p % 16 == 0) else "dve")))
                if len(l3q) >= 8:
                    for prev_p, prev_h2 in l3q[:4]:
                        emit_l3(prev_p, prev_h2)
                    l3q = l3q[4:]
                l2q += [4 * b + q for q in range(4)]
            for p in l2q:
                z2 = emit_l2(p, h1s.pop(2 * p), h1s.pop(2 * p + 1))
                l3q.append((p, emit_h2(p, z2, "act" if (p % 2 == 1 or p % 16 == 0) else "dve")))
            for prev_p, prev_h2 in l3q:
                emit_l3(prev_p, prev_h2)

            # ---- drain partial rows to SBUF, then DRAM; host sums them
            nrow = 32 * (l3_pos - 1) + 1
            srow = srp.tile([nrow, BT], F32, tag="srow")
            nc.scalar.activation(srow, pout[0:nrow, :], Copy)
            for k in range(l3_pos):
                nc.sync.dma_start(
                    out=out_d[l3_pos * bt + k : l3_pos * bt + k + 1, :],
                    in_=srow[32 * k : 32 * k + 1, :],
                )

    nc.compile()
    return nc


def _prep_shared(W1, b1, W2, b2, W3):
    import ml_dtypes

    bf = ml_dtypes.bfloat16
    w1q = W1.reshape(32, 4, S).transpose(1, 0, 2).reshape(4, 32 * S)
    b1t = np.ascontiguousarray(b1.T)  # [S, F]
    w2t = W2.transpose(1, 0, 2).reshape(S, F * H1)
    b2pm = np.empty((2 * H1, NPAIR), np.float32)
    w3pm = np.empty((2 * H1, NPAIR), np.float32)
    W3f = W3.reshape(F, H1)
    for p in range(NPAIR):
        b2pm[:H1, p] = b2[2 * p]
        b2pm[H1:, p] = b2[2 * p + 1]
        w3pm[:H1, p] = W3f[2 * p]
        w3pm[H1:, p] = W3f[2 * p + 1]
    return {
        "w1q": np.ascontiguousarray(w1q).astype(bf),
        "b1t": b1t.astype(np.float32),
        "w2t": np.ascontiguousarray(w2t).astype(bf),
        "b2p": b2pm,
        "w3p": w3pm.astype(bf),
    }


def _prep_core_inputs(xc, shared):
    import ml_dtypes

    m = dict(shared)
    xT = np.ascontiguousarray(xc.T)  # [F, BLOC]
    m["xTg"] = np.ascontiguousarray(
        xT.reshape(32, 4, BLOC).transpose(1, 0, 2)
    ).astype(ml_dtypes.bfloat16)
    return m


def kernel(x, W1, b1, W2, b2, W3, b3, bias, _trace=False):
    x = np.asarray(x, np.float32)
    W1 = np.asarray(W1, np.float32)
    b1 = np.asarray(b1, np.float32)
    W2 = np.asarray(W2, np.float32)
    b2 = np.asarray(b2, np.float32)
    W3 = np.asarray(W3, np.float32)
    b3 = np.asarray(b3, np.float32)
    bias = np.asarray(bias, np.float32)

    if "nc" not in _CACHE:
        _CACHE["nc"] = _build()
    nc = _CACHE["nc"]

    shared = _prep_shared(W1, b1, W2, b2, W3)
    in_maps = [
        _prep_core_inputs(x[c * BLOC : (c + 1) * BLOC], shared)
        for c in range(NCORES)
    ]

    res = run_bass_kernel_spmd(
        nc, in_maps, core_ids=list(range(NCORES)), trace=_trace
    )
    _CACHE["last_result"] = res

    const = float(b3.sum()) + float(bias.reshape(-1)[0])
    parts = []
    for c in range(NCORES):
        o = res.results[c]["out"]  # [NBT*L3_POS, BT]
        parts.append(o.reshape(NBT, L3_POS, BT).sum(axis=1).reshape(BLOC))
    out = np.concatenate(parts) + const
    return out.reshape(B, 1).astype(np.float32)
